# revision 44
# baseline (speedup 1.0000x reference)
"""Distributed attention kernel for 8 TRN2 NeuronCores.

Problem: cross-attention (q from target, k/v from reference) with
B=2, N=M=2048, C=1024, H=16 heads, hd=64, followed by an output
projection with bias.

Sharding (data + head parallel):
  core c in 0..7 owns heads {2c, 2c+1} for BOTH batches. It computes
  K^T/Q^T/V for its heads and attention (softmax over keys), producing
  UNNORMALIZED x_local^T [128ch, 2048m] per batch plus per-head softmax
  denominators. One AllToAll PER BATCH redistributes [130, 256] blocks
  (128 channels + 2 denominator rows) so core c owns output rows
  [c*256, (c+1)*256) with ALL 1024 channels; core c then normalizes
  (one reciprocal + broadcast-multiply per batch) and applies the full
  Wproj ([1024,1024], replicated) + bias to its row-block.

Schedule: K^T and the first half of Q^T are computed first so the
softmax exp stream (the ScalarE roofline of this kernel, ~147us) starts
as early as possible; V, the rest of Q, and ALL of batch-1's QKV are
emitted as "fillers" between attention kc-pairs so the PE does them in
the gaps while ACT streams exps back-to-back.

Queue discipline (3 DMA-trigger queues: sync, scalar, gpsimd):
  - scalar carries only pre-attention input loads, then exps: nothing
    pool- or collective-gated may sit in front of an exp.
  - sync carries input chunks + per-m-tile staging into the a2a input.
  - gpsimd carries input chunks, the collective triggers, and all
    proj-side loads (which wait on the collectives) so a slow AllToAll
    can never head-of-line-block the attention/staging path.

Matmuls run in bf16 (f32 PSUM accumulation); softmax denominators come
free as a ones-column appended to V.
"""

import functools

import numpy as np

B = 2
N = 2048  # reference rows (keys)
M = 2048  # target rows (queries)
C = 1024
H = 16
HD = 64
NCORES = 8
HPC = 2  # heads per core
CHPC = HPC * HD  # 128 channels per core
CHP2 = CHPC + 2  # + 2 denominator rows in the a2a payload
MBLK = M // NCORES  # 256 output rows owned per core (per batch)
MT = 512  # attention m-tile
KC = N // 128  # 16 key chunks
CC = C // 128  # 8 contraction chunks
NMT = M // MT  # 4 m-tiles per batch
HS = MBLK // 2  # 128-row half-slot


@functools.lru_cache(maxsize=1)
def _build():
    import concourse.bacc as bacc
    import concourse.mybir as mybir
    import concourse.tile as tile

    fp32 = mybir.dt.float32
    bf16 = mybir.dt.bfloat16
    AF = mybir.ActivationFunctionType

    nc = bacc.Bacc("TRN2", target_bir_lowering=False, debug=False, num_devices=NCORES)

    xrefT = nc.dram_tensor("xrefT", [B, C, N], bf16, kind="ExternalInput")
    xtgtT = nc.dram_tensor("xtgtT", [B, C, M], bf16, kind="ExternalInput")
    wq = nc.dram_tensor("wq", [C, CHPC], bf16, kind="ExternalInput")
    wk = nc.dram_tensor("wk", [C, CHPC], bf16, kind="ExternalInput")
    wv = nc.dram_tensor("wv", [C, CHPC], bf16, kind="ExternalInput")
    wproj = nc.dram_tensor("wproj", [C, C], bf16, kind="ExternalInput")
    bproj = nc.dram_tensor("bproj", [C], fp32, kind="ExternalInput")
    out = nc.dram_tensor("out", [B, C, MBLK], fp32, kind="ExternalOutput")

    with tile.TileContext(nc) as tc:
        with (
            tc.tile_pool(name="wpool", bufs=1) as wpool,
            tc.tile_pool(name="xr", bufs=10) as xrpool,
            tc.tile_pool(name="xt", bufs=10) as xtpool,
            tc.tile_pool(name="kqv", bufs=1) as kqv,
            tc.tile_pool(name="epool", bufs=10) as epool,
            tc.tile_pool(name="spool", bufs=4) as spool,
            tc.tile_pool(name="ppool", bufs=2) as ppool,
            tc.tile_pool(name="psA", bufs=3, space="PSUM") as psA,
            tc.tile_pool(name="psO", bufs=2, space="PSUM") as psO,
            tc.tile_pool(name="dram", bufs=1, space="DRAM") as dpool,
        ):
            # ---- PE warm-up: ~48 matmuls on an uninitialized scratch ----
            # tile (results unread).  They stream while the input DMAs ramp,
            # so the HAM clock-gate is already at 2.4 GHz when the first
            # real QKV matmul arrives (~20us in) instead of half-clock.
            scratch = wpool.tile([128, 5 * 128], bf16, name="warmscratch")
            nc.vector.memset(scratch[:], 0.0)
            for wg in range(3):
                wps = psA.tile([128, 2 * MT], fp32, tag="big", name=f"wu{wg}")
                for wi in range(16):
                    nc.tensor.matmul(
                        wps[:, 0:MT],
                        lhsT=scratch[:, 0:128],
                        rhs=scratch[:, 128:5 * 128],
                        start=(wi == 0),
                        stop=(wi == 15),
                    )

            # ---- weight loads (emitted first; DMA engines run ahead) ----
            wq_sb = wpool.tile([128, CC, CHPC], bf16)
            wk_sb = wpool.tile([128, CC, CHPC], bf16)
            wv_sb = wpool.tile([128, CC, CHPC], bf16)
            for cc in range(CC):
                nc.sync.dma_start(wk_sb[:, cc, :], wk[cc * 128:(cc + 1) * 128, :])
                nc.gpsimd.dma_start(wv_sb[:, cc, :], wv[cc * 128:(cc + 1) * 128, :])
                nc.scalar.dma_start(wq_sb[:, cc, :], wq[cc * 128:(cc + 1) * 128, :])

            kT = [kqv.tile([128, N], bf16, tag=f"kT{b}", name=f"kT{b}") for b in range(B)]
            qT = [kqv.tile([128, M], bf16, tag=f"qT{b}", name=f"qT{b}") for b in range(B)]
            vA = [
                kqv.tile([128, KC, HPC, HD + 1], bf16, tag=f"vA{b}", name=f"vA{b}")
                for b in range(B)
            ]
            oU = [kqv.tile([128, M], bf16, tag=f"oU{b}", name=f"oU{b}") for b in range(B)]
            for b in range(B):
                nc.vector.memset(vA[b][:, :, :, HD:HD + 1], 1.0)

            a2a_in = [
                dpool.tile([NCORES, CHP2, MBLK], bf16, tag=f"a2a_in{b}", name=f"a2a_in{b}")
                for b in range(B)
            ]
            a2a_out = [
                dpool.tile([NCORES, CHP2, MBLK], bf16, tag=f"a2a_out{b}", name=f"a2a_out{b}")
                for b in range(B)
            ]
            # per-batch bounce for the reciprocal'd denominators, indexed
            # [h-within-pair, src-core, m]: row (hh, i) is head 2i+hh, so a
            # [1, 8, m] slice broadcasts across each 64-partition head group
            rdram = dpool.tile([B, 2, NCORES, MBLK], bf16, tag="rdram", name="rdram")

            xch = {}  # (tensor, b, cc) -> sbuf chunk tile

            def load_chunk(which, b, cc, half=None, eng=None):
                src = xrefT if which == "r" else xtgtT
                pool = xrpool if which == "r" else xtpool
                key = (which, b, cc)
                if key in xch:
                    t = xch[key]
                else:
                    t = pool.tile([128, N], bf16, tag="x", name=f"x{which}{b}_{cc}")
                    xch[key] = t
                if half is None:
                    eng.dma_start(t[:], src[b, cc * 128:(cc + 1) * 128, :])
                else:
                    cols = slice(half * (N // 2), (half + 1) * (N // 2))
                    eng.dma_start(t[:, cols], src[b, cc * 128:(cc + 1) * 128, cols])

            def kt_part(b, nt, w_sb, dstT, which):
                # one 512-col slice of a K^T/Q^T projection: 8 MMs + 1 copy
                ps = psA.tile([128, 2 * MT], fp32, tag="big", name=f"kp{which}{b}{nt}")
                for cc in range(CC):
                    nc.tensor.matmul(
                        ps[:, 0:MT],
                        lhsT=w_sb[:, cc, :],
                        rhs=xch[(which, b, cc)][:, nt * MT:(nt + 1) * MT],
                        start=(cc == 0),
                        stop=(cc == CC - 1),
                    )
                nc.vector.tensor_copy(dstT[:, nt * MT:(nt + 1) * MT], ps[:, 0:MT])

            def v_part(b, q):
                # V rows for key chunks 2q, 2q+1: 16 MMs + 1 copy
                ps = psA.tile([128, 2 * MT], fp32, tag="big", name=f"vp{b}{q}")
                for j in range(2):
                    kc = 2 * q + j
                    for cc in range(CC):
                        nc.tensor.matmul(
                            ps[:, j * 128:(j + 1) * 128],
                            lhsT=xch[("r", b, cc)][:, kc * 128:(kc + 1) * 128],
                            rhs=wv_sb[:, cc, :],
                            start=(cc == 0),
                            stop=(cc == CC - 1),
                        )
                nc.vector.tensor_copy(
                    vA[b][:, 2 * q:2 * q + 2, :, 0:HD],
                    ps[:, 0:256].rearrange("p (k h d) -> p k h d", k=2, h=HPC),
                )

            scale = float(HD) ** -0.5

            def attn_mt(b, mt, fillers=None, av_lag=1, tail=(), scalar_cast=False,
                        defer_finish=False):
                fillers = fillers or {}
                # for the LAST m-tile the exp stream is over, so its output
                # casts can run on the then-idle scalar engine, shortening
                # the last-exp -> a2a-fence chain (vector would serialize)
                cast = nc.scalar.copy if scalar_cast else nc.vector.tensor_copy
                po = [
                    psO.tile([HD + 1, MT], fp32, tag="o", name=f"po{h}")
                    for h in range(HPC)
                ]

                def av_pair(kc, eS):
                    for h in range(HPC):
                        for j in range(2):
                            nc.tensor.matmul(
                                po[h][:],
                                lhsT=vA[b][:, kc + j, h, :],
                                rhs=eS[h][:, j, :],
                                start=(kc == 0 and j == 0),
                                stop=(kc == KC - 2 and j == 1),
                            )

                # software-pipelined by av_lag kc-pairs: the AV of pair k is
                # emitted AFTER the S^T/exp of pair k+av_lag (and any filler
                # PE work), so the PE always has wait-free work while the ACT
                # engine streams exps back-to-back (ACT is the bottleneck).
                pending = []
                for pi, kc in enumerate(range(0, KC, 2)):
                    pss = [
                        psA.tile([128, 2 * MT], fp32, tag="big", name="pss")
                        for _ in range(HPC)
                    ]
                    for j in range(2):
                        # the two heads sit at partitions 0-63 / 64-127 so the
                        # PE row-groups run their K=64 matmuls concurrently
                        for h in range(HPC):
                            nc.tensor.matmul(
                                pss[h][:, j * MT:(j + 1) * MT],
                                lhsT=kT[b][h * HD:(h + 1) * HD, (kc + j) * 128:(kc + j + 1) * 128],
                                rhs=qT[b][h * HD:(h + 1) * HD, mt * MT:(mt + 1) * MT],
                                start=True,
                                stop=True,
                            )
                    eS = [
                        epool.tile([128, 2, MT], bf16, tag="eS", name="eS")
                        for _ in range(HPC)
                    ]
                    for h in range(HPC):
                        nc.scalar.activation(
                            eS[h][:].rearrange("p a b -> p (a b)"),
                            pss[h][:],
                            AF.Exp,
                            scale=scale,
                        )
                    for f in fillers.get(pi, ()):
                        f()
                    if len(pending) == av_lag:
                        av_pair(*pending.pop(0))
                    pending.append((kc, eS))
                ti = 0
                while len(pending) > 1:
                    if ti < len(tail):
                        tail[ti]()
                        ti += 1
                    av_pair(*pending.pop(0))
                for f in tail[ti:]:
                    f()

                # final pair: per-head AV with the output/denominator casts
                # interleaved, so staging starts while the other head's AV
                # still runs (shortens the last-exp -> a2a-fence chain).
                # m-tile mt covers dst cores (mt%2)*4+q at column-half mt//2;
                # staging triggers alternate sync/gpsimd so the last m-tile's
                # six triggers don't serialize ~4us ahead of the a2a fence.
                hf = mt // 2
                s0 = (mt % 2) * 4
                kc_l, eS_l = pending.pop(0)

                # With defer_finish this block is returned as a closure and
                # emitted as the NEXT m-tile's pair-0 filler: scores do not
                # touch po, so the exp stream runs straight across the m-tile
                # boundary instead of stalling ~1.8us behind these AVs.
                def finish():
                    for h in range(HPC):
                        for j in range(2):
                            nc.tensor.matmul(
                                po[h][:],
                                lhsT=vA[b][:, kc_l + j, h, :],
                                rhs=eS_l[h][:, j, :],
                                start=(kc_l == 0 and j == 0),
                                stop=(kc_l == KC - 2 and j == 1),
                            )
                        cast(
                            oU[b][h * HD:(h + 1) * HD, mt * MT:(mt + 1) * MT],
                            po[h][0:HD, :],
                        )
                        dt = spool.tile([HD + 1, MT], bf16, tag="dt", name="dt")
                        cast(dt[HD:HD + 1, :], po[h][HD:HD + 1, :])
                        [nc.sync, nc.gpsimd][h].dma_start(
                            a2a_in[b][s0:s0 + 4, CHPC + h:CHPC + h + 1, hf * HS:(hf + 1) * HS],
                            dt[HD:HD + 1, :].rearrange("a (q c) -> a q c", q=4),
                        )
                    # the quarters span BOTH heads' partition rows, so they
                    # can only be staged after the second head's oU cast
                    for q in range(4):
                        [nc.sync, nc.gpsimd][q % 2].dma_start(
                            a2a_in[b][s0 + q][0:CHPC, hf * HS:(hf + 1) * HS],
                            oU[b][:, mt * MT + q * HS:mt * MT + (q + 1) * HS],
                        )

                if defer_finish:
                    return finish
                finish()

            def fire_a2a(b):
                nc.gpsimd.collective_compute(
                    "AllToAll",
                    mybir.AluOpType.bypass,
                    replica_groups=[list(range(NCORES))],
                    ins=[a2a_in[b][:].opt()],
                    outs=[a2a_out[b][:].opt()],
                )

            def proj_load(b, ldeng):
                # dsb rows 0-7 = even heads (h=0 of each src core), 8-15 = odd.
                # Allocated from the attention-path "dt" tag ON PURPOSE: the
                # pool WAR forces this load (and the reciprocal chain behind
                # it) after the attention dt casts, so the Tile scheduler can
                # never interleave a collective-gated wait in front of the
                # attention work on the vector queue.
                dsb = spool.tile([HD + 1, MT], bf16, tag="dt", name=f"dsb{b}")
                dsf = dsb[:].bitcast(fp32)  # fp32 view: gpsimd DMA casts on load
                for hh in range(2):
                    ldeng.dma_start(
                        dsf[hh * NCORES:(hh + 1) * NCORES, 0:MBLK],
                        a2a_out[b][:, CHPC + hh:CHPC + hh + 1, :].rearrange(
                            "i h m -> i (h m)"
                        ),
                    )
                # gather my m-block (all 1024 channels) in ONE trigger: the
                # permuted DRAM access pattern costs the same descriptors as
                # eight per-slot loads but 7 fewer ~650ns queue slots
                y_sb = ppool.tile([128, NCORES, MBLK], bf16, tag="y", name=f"y{b}")
                ldeng.dma_start(
                    y_sb[:],
                    a2a_out[b][:, 0:CHPC, :].rearrange("s p m -> p s m"),
                )
                rf = ppool.tile([16, MBLK], fp32, tag="rf", name=f"rf{b}")
                # ~51-ULP fast reciprocal (~5x cheaper than the iterative
                # divide); denominators are safely in [1, ~1e4]
                nc.vector.reciprocal_approx_fast(rf[:], dsf[0:16, 0:MBLK])
                rN = ppool.tile([16, MBLK], bf16, tag="rN", name=f"rN{b}")
                nc.vector.tensor_copy(rN[:], rf[:])
                ldeng.dma_start(
                    rdram[b].rearrange("h i m -> (h i) m"), rN[:]
                )
                rb = ppool.tile([128, NCORES, MBLK], bf16, tag="rb", name=f"rb{b}")
                for hh in range(2):
                    ldeng.dma_start(
                        rb[hh * HD:(hh + 1) * HD],
                        rdram[b, hh:hh + 1].to_broadcast((HD, NCORES, MBLK)),
                    )
                xn = ppool.tile([128, NCORES, MBLK], bf16, tag="xn", name=f"xn{b}")
                # two halves: the proj matmuls' cc 0-3 start on the first
                # half while the second is still multiplying
                for g in range(2):
                    nc.vector.tensor_mul(
                        xn[:, 4 * g:4 * g + 4, :],
                        y_sb[:, 4 * g:4 * g + 4, :],
                        rb[:, 4 * g:4 * g + 4, :],
                    )
                return xn

            def proj_mm(b, xn):
                for oc in range(CC):
                    psb = psA.tile([128, 2 * MT], fp32, tag="big", name="pp")
                    ps = psb[:, 0:MBLK]
                    for cc in range(CC):
                        nc.tensor.matmul(
                            ps[:],
                            lhsT=wp_sb[:, cc, oc * 128:(oc + 1) * 128],
                            rhs=xn[:, cc, :],
                            start=(cc == 0),
                            stop=(cc == CC - 1),
                        )
                    osb = ppool.tile([128, MBLK], fp32, tag="outsb", name="osb")
                    nc.scalar.activation(
                        osb[:], ps[:], AF.Identity, bias=bias_sb[:, oc:oc + 1]
                    )
                    nc.sync.dma_start(out[b, oc * 128:(oc + 1) * 128, :], osb[:])

            # ================= emission schedule =================
            # batch-0 loads, halves-first so kt parts start after ~2MB of DMA
            E3 = [nc.sync, nc.gpsimd, nc.scalar]
            for half in range(2):
                for cc in range(CC):
                    load_chunk("r", 0, cc, half=half, eng=E3[cc % 3])
            for half in range(2):
                for cc in range(CC):
                    load_chunk("t", 0, cc, half=half, eng=E3[cc % 3])
            # wproj/bias after the batch-0 chunks on the scalar queue; needed
            # only by proj(0) mid-kernel
            wp_sb = wpool.tile([128, CC, C], bf16, name="wp_sb")
            for cc in range(CC):
                nc.scalar.dma_start(wp_sb[:, cc, :], wproj[cc * 128:(cc + 1) * 128, :])
            bias_sb = wpool.tile([128, CC], fp32, name="bias_sb")
            nc.scalar.dma_start(bias_sb[:], bproj.ap().rearrange("(a p) -> p a", p=128))

            # the minimum PE work before attention m-tile 0 can start: its
            # pairs 0-3 only need K^T keys 0-1023 and Q^T cols 0-511
            kt_part(0, 0, wk_sb, kT[0], "r")
            kt_part(0, 0, wq_sb, qT[0], "t")

            # everything else rides in attention-pair filler slots; av_lag=2
            # gives the just-in-time V parts one pair of slack
            P = functools.partial
            f_prev = attn_mt(
                0, 0,
                fillers={
                    0: (P(kt_part, 0, 1, wk_sb, kT[0], "r"),),
                    1: (P(kt_part, 0, 2, wk_sb, kT[0], "r"),),
                    2: (P(kt_part, 0, 3, wk_sb, kT[0], "r"),),
                    3: (P(kt_part, 0, 1, wq_sb, qT[0], "t"),),
                    **{pi: (P(v_part, 0, pi - 4),) for pi in range(4, 8)},
                },
                av_lag=4,
                tail=(
                    P(v_part, 0, 4),
                    P(v_part, 0, 5),
                    P(v_part, 0, 6),
                    P(v_part, 0, 7),
                ),
                defer_finish=True,
            )
            # batch-1 chunk loads: emitted only now so their pool-recycling
            # waits (on r0/t0 release) sit behind this m-tile's staging in
            # the sync/gpsimd queues (scalar stays exp-only)
            E2 = [nc.sync, nc.gpsimd]
            for cc in range(CC):
                load_chunk("r", 1, cc, eng=E2[cc % 2])
            f_prev = attn_mt(0, 1, {
                0: (f_prev, functools.partial(kt_part, 0, 2, wq_sb, qT[0], "t")),
                2: (functools.partial(kt_part, 0, 3, wq_sb, qT[0], "t"),),
            }, defer_finish=True)
            for cc in range(CC):
                load_chunk("t", 1, cc, eng=E2[cc % 2])
            f_prev = attn_mt(0, 2, {
                0: (f_prev, functools.partial(kt_part, 1, 0, wk_sb, kT[1], "r")),
                **{pi: (functools.partial(kt_part, 1, pi, wk_sb, kT[1], "r"),)
                   for pi in range(1, 4)},
            }, defer_finish=True)
            attn_mt(0, 3, {
                0: (f_prev, functools.partial(kt_part, 1, 0, wq_sb, qT[1], "t")),
                2: (functools.partial(kt_part, 1, 1, wq_sb, qT[1], "t"),),
            })
            fire_a2a(0)
            f_prev = attn_mt(
                1, 0, {pi: (functools.partial(v_part, 1, pi),) for pi in range(8)},
                defer_finish=True,
            )
            f_prev = attn_mt(1, 1, {
                0: (f_prev, functools.partial(kt_part, 1, 2, wq_sb, qT[1], "t")),
                2: (functools.partial(kt_part, 1, 3, wq_sb, qT[1], "t"),),
            }, defer_finish=True)
            f_prev = attn_mt(1, 2, {0: (f_prev,)}, defer_finish=True)
            # proj(0)'s loads + reciprocal chain are emitted here so their
            # gpsimd triggers sit BEFORE the a2a(1) trigger (whose fence
            # would otherwise hold them hostage until all staging lands);
            # the dsb pool-WAR still pins the vector chain safely behind
            # attn(1,1)'s dt casts.  Its MATMULS stay after attn(1,3) so
            # the PE queue is never head-of-line blocked -- and they double
            # as the HAM warm-keeper spanning the a2a(1) wait.
            xn0 = proj_load(0, nc.gpsimd)
            attn_mt(1, 3, {0: (f_prev,)}, scalar_cast=True)
            fire_a2a(1)
            proj_mm(0, xn0)
            xn1 = proj_load(1, nc.gpsimd)
            # a short HAM warm-keeper after proj(0): spans the early part of
            # the a2a(1) wait so proj(1)'s matmuls are less likely to run at
            # the re-throttled clock; sized to finish before xn1 is ready
            # even when the collective is fast.
            for wg in range(4):
                wps = psA.tile([128, 2 * MT], fp32, tag="big", name=f"warm{wg}")
                for wi in range(16):
                    nc.tensor.matmul(
                        wps[:, 0:MT],
                        lhsT=wk_sb[:, wi % CC, :],
                        rhs=qT[1][:, 0:MT],
                        start=(wi == 0),
                        stop=(wi == 15),
                    )
            proj_mm(1, xn1)

    nc.compile()
    return nc


def _shard_inputs(reference_data, target_data, Wq, Wkv, Wproj, bproj):
    import ml_dtypes

    bf16 = ml_dtypes.bfloat16
    xrefT = np.ascontiguousarray(
        np.asarray(reference_data, dtype=np.float32).transpose(0, 2, 1)
    ).astype(bf16)
    xtgtT = np.ascontiguousarray(
        np.asarray(target_data, dtype=np.float32).transpose(0, 2, 1)
    ).astype(bf16)
    Wq = np.asarray(Wq, dtype=np.float32)
    Wkv = np.asarray(Wkv, dtype=np.float32)
    Wproj_b = np.asarray(Wproj, dtype=np.float32).astype(bf16)
    bproj = np.asarray(bproj, dtype=np.float32)

    in_maps = []
    for c in range(NCORES):
        lo, hi = c * CHPC, (c + 1) * CHPC
        in_maps.append(
            {
                "xrefT": xrefT,
                "xtgtT": xtgtT,
                "wq": Wq[:, lo:hi].astype(bf16),
                "wk": Wkv[:, lo:hi].astype(bf16),
                "wv": Wkv[:, C + lo:C + hi].astype(bf16),
                "wproj": Wproj_b,
                "bproj": bproj,
            }
        )
    return in_maps


def _ensure_ntff_hook():
    """Register the axon NTFF profile hook if the image's antenv lacks it."""
    try:
        import antenv.axon_hooks  # noqa: F401

        return
    except ImportError:
        pass
    import sys
    import types

    import antenv

    mod = types.ModuleType("antenv.axon_hooks")
    state = {"hook": None}
    mod.set_axon_ntff_profile_hook = lambda h: state.__setitem__("hook", h)
    mod.get_axon_ntff_profile_hook = lambda: state["hook"]
    sys.modules["antenv.axon_hooks"] = mod
    antenv.axon_hooks = mod
    try:
        from trn_agent_boot.trn_boot import _ntff_profile_via_ctypes

        mod.set_axon_ntff_profile_hook(
            _ntff_profile_via_ctypes("/opt/axon/libaxon_pjrt.so")
        )
    except Exception:
        pass


def run(inputs: dict, trace: bool = False):
    """Compile (cached), run on 8 cores, return (full_output, BassKernelResults)."""
    from concourse.bass_utils import run_bass_kernel_spmd

    if trace:
        _ensure_ntff_hook()
    nc = _build()
    in_maps = _shard_inputs(**inputs)
    res = run_bass_kernel_spmd(
        nc, in_maps, core_ids=list(range(NCORES)), trace=trace
    )
    return _assemble(res), res


def _assemble(res):
    full = np.zeros((B, M, C), dtype=np.float32)
    hs = MBLK // 2
    for c in range(NCORES):
        blk = np.asarray(res.results[c]["out"], dtype=np.float32)  # [B, C, MBLK]
        for b in range(B):
            for hf in range(2):
                full[b, 1024 * hf + c * hs:1024 * hf + (c + 1) * hs, :] = (
                    blk[b][:, hf * hs:(hf + 1) * hs].T
                )
    return full


def kernel(reference_data, target_data, Wq, Wkv, Wproj, bproj) -> np.ndarray:
    full, _ = run(
        {
            "reference_data": reference_data,
            "target_data": target_data,
            "Wq": Wq,
            "Wkv": Wkv,
            "Wproj": Wproj,
            "bproj": bproj,
        }
    )
    return full


# revision 46
# speedup vs baseline: 1.0365x; 1.0365x over previous
"""Distributed attention kernel for 8 TRN2 NeuronCores.

Problem: cross-attention (q from target, k/v from reference) with
B=2, N=M=2048, C=1024, H=16 heads, hd=64, followed by an output
projection with bias.

Sharding (data + head parallel):
  core c in 0..7 owns heads {2c, 2c+1} for BOTH batches. It computes
  K^T/Q^T/V for its heads and attention (softmax over keys), producing
  UNNORMALIZED x_local^T [128ch, 2048m] per batch plus per-head softmax
  denominators. One AllToAll PER BATCH redistributes [130, 256] blocks
  (128 channels + 2 denominator rows) so core c owns output rows
  [c*256, (c+1)*256) with ALL 1024 channels; core c then normalizes
  (one reciprocal + broadcast-multiply per batch) and applies the full
  Wproj ([1024,1024], replicated) + bias to its row-block.

Schedule: K^T and the first half of Q^T are computed first so the
softmax exp stream (the ScalarE roofline of this kernel, ~147us) starts
as early as possible; V, the rest of Q, and ALL of batch-1's QKV are
emitted as "fillers" between attention kc-pairs so the PE does them in
the gaps while ACT streams exps back-to-back.

Queue discipline (3 DMA-trigger queues: sync, scalar, gpsimd):
  - scalar carries only pre-attention input loads, then exps: nothing
    pool- or collective-gated may sit in front of an exp.
  - sync carries input chunks + per-m-tile staging into the a2a input.
  - gpsimd carries input chunks, the collective triggers, and all
    proj-side loads (which wait on the collectives) so a slow AllToAll
    can never head-of-line-block the attention/staging path.

Matmuls run in bf16 (f32 PSUM accumulation); softmax denominators come
free as a ones-column appended to V.
"""

import functools

import numpy as np

B = 2
N = 2048  # reference rows (keys)
M = 2048  # target rows (queries)
C = 1024
H = 16
HD = 64
NCORES = 8
HPC = 2  # heads per core
CHPC = HPC * HD  # 128 channels per core
CHP2 = CHPC + 2  # + 2 denominator rows in the a2a payload
MBLK = M // NCORES  # 256 output rows owned per core (per batch)
MT = 512  # attention m-tile
KC = N // 128  # 16 key chunks
CC = C // 128  # 8 contraction chunks
NMT = M // MT  # 4 m-tiles per batch
HS = MBLK // 2  # 128-row half-slot


@functools.lru_cache(maxsize=1)
def _build():
    import concourse.bacc as bacc
    import concourse.mybir as mybir
    import concourse.tile as tile

    fp32 = mybir.dt.float32
    bf16 = mybir.dt.bfloat16
    AF = mybir.ActivationFunctionType

    nc = bacc.Bacc("TRN2", target_bir_lowering=False, debug=False, num_devices=NCORES)

    xrefT = nc.dram_tensor("xrefT", [B, C, N], bf16, kind="ExternalInput")
    xtgtT = nc.dram_tensor("xtgtT", [B, C, M], bf16, kind="ExternalInput")
    wq = nc.dram_tensor("wq", [C, CHPC], bf16, kind="ExternalInput")
    wk = nc.dram_tensor("wk", [C, CHPC], bf16, kind="ExternalInput")
    wv = nc.dram_tensor("wv", [C, CHPC], bf16, kind="ExternalInput")
    wproj = nc.dram_tensor("wproj", [C, C], bf16, kind="ExternalInput")
    bproj = nc.dram_tensor("bproj", [C], fp32, kind="ExternalInput")
    out = nc.dram_tensor("out", [B, C, MBLK], fp32, kind="ExternalOutput")

    with tile.TileContext(nc) as tc:
        with (
            tc.tile_pool(name="wpool", bufs=1) as wpool,
            tc.tile_pool(name="xr", bufs=10) as xrpool,
            tc.tile_pool(name="xt", bufs=10) as xtpool,
            tc.tile_pool(name="kqv", bufs=1) as kqv,
            tc.tile_pool(name="epool", bufs=10) as epool,
            tc.tile_pool(name="spool", bufs=4) as spool,
            tc.tile_pool(name="ppool", bufs=2) as ppool,
            tc.tile_pool(name="psA", bufs=3, space="PSUM") as psA,
            tc.tile_pool(name="psO", bufs=2, space="PSUM") as psO,
            tc.tile_pool(name="dram", bufs=1, space="DRAM") as dpool,
        ):
            # ---- PE warm-up: ~48 matmuls on an uninitialized scratch ----
            # tile (results unread).  They stream while the input DMAs ramp,
            # so the HAM clock-gate is already at 2.4 GHz when the first
            # real QKV matmul arrives (~20us in) instead of half-clock.
            scratch = wpool.tile([128, 5 * 128], bf16, name="warmscratch")
            nc.vector.memset(scratch[:], 0.0)
            for wg in range(3):
                wps = psA.tile([128, 2 * MT], fp32, tag="big", name=f"wu{wg}")
                for wi in range(16):
                    nc.tensor.matmul(
                        wps[:, 0:MT],
                        lhsT=scratch[:, 0:128],
                        rhs=scratch[:, 128:5 * 128],
                        start=(wi == 0),
                        stop=(wi == 15),
                    )

            # ---- weight loads (emitted first; DMA engines run ahead) ----
            wq_sb = wpool.tile([128, CC, CHPC], bf16)
            wk_sb = wpool.tile([128, CC, CHPC], bf16)
            wv_sb = wpool.tile([128, CC, CHPC], bf16)
            for cc in range(CC):
                nc.sync.dma_start(wk_sb[:, cc, :], wk[cc * 128:(cc + 1) * 128, :])
                nc.gpsimd.dma_start(wv_sb[:, cc, :], wv[cc * 128:(cc + 1) * 128, :])
                nc.scalar.dma_start(wq_sb[:, cc, :], wq[cc * 128:(cc + 1) * 128, :])

            kT = [kqv.tile([128, N], bf16, tag=f"kT{b}", name=f"kT{b}") for b in range(B)]
            qT = [kqv.tile([128, M], bf16, tag=f"qT{b}", name=f"qT{b}") for b in range(B)]
            vA = [
                kqv.tile([128, KC, HPC, HD + 1], bf16, tag=f"vA{b}", name=f"vA{b}")
                for b in range(B)
            ]
            oU = [kqv.tile([128, M], bf16, tag=f"oU{b}", name=f"oU{b}") for b in range(B)]
            for b in range(B):
                nc.vector.memset(vA[b][:, :, :, HD:HD + 1], 1.0)

            a2a_in = [
                dpool.tile([NCORES, CHP2, MBLK], bf16, tag=f"a2a_in{b}", name=f"a2a_in{b}")
                for b in range(B)
            ]
            a2a_out = [
                dpool.tile([NCORES, CHP2, MBLK], bf16, tag=f"a2a_out{b}", name=f"a2a_out{b}")
                for b in range(B)
            ]
            # per-batch bounce for the reciprocal'd denominators, indexed
            # [h-within-pair, src-core, m]: row (hh, i) is head 2i+hh, so a
            # [1, 8, m] slice broadcasts across each 64-partition head group
            rdram = dpool.tile([B, 2, NCORES, MBLK], bf16, tag="rdram", name="rdram")

            xch = {}  # (tensor, b, cc) -> sbuf chunk tile

            def load_chunk(which, b, cc, half=None, eng=None):
                src = xrefT if which == "r" else xtgtT
                pool = xrpool if which == "r" else xtpool
                key = (which, b, cc)
                if key in xch:
                    t = xch[key]
                else:
                    t = pool.tile([128, N], bf16, tag="x", name=f"x{which}{b}_{cc}")
                    xch[key] = t
                if half is None:
                    eng.dma_start(t[:], src[b, cc * 128:(cc + 1) * 128, :])
                else:
                    cols = slice(half * (N // 2), (half + 1) * (N // 2))
                    eng.dma_start(t[:, cols], src[b, cc * 128:(cc + 1) * 128, cols])

            def kt_part(b, nt, w_sb, dstT, which, warm=0):
                # one 512-col slice of a K^T/Q^T projection: 8 MMs + 1 copy.
                # warm>0 interleaves that many scratch matmuls after each
                # DMA-gated chunk matmul: the PE queue is in-order, so these
                # are what keeps the HAM clock-gate busy across the ~1-2us
                # chunk-arrival gaps during the lead-in.
                ps = psA.tile([128, 2 * MT], fp32, tag="big", name=f"kp{which}{b}{nt}")
                wps = (
                    psA.tile([128, 2 * MT], fp32, tag="big", name=f"kw{which}{b}{nt}")
                    if warm else None
                )
                for cc in range(CC):
                    nc.tensor.matmul(
                        ps[:, 0:MT],
                        lhsT=w_sb[:, cc, :],
                        rhs=xch[(which, b, cc)][:, nt * MT:(nt + 1) * MT],
                        start=(cc == 0),
                        stop=(cc == CC - 1),
                    )
                    for _ in range(warm):
                        nc.tensor.matmul(
                            wps[:, 0:MT],
                            lhsT=scratch[:, 0:128],
                            rhs=scratch[:, 128:5 * 128],
                            start=True,
                            stop=True,
                        )
                nc.vector.tensor_copy(dstT[:, nt * MT:(nt + 1) * MT], ps[:, 0:MT])

            def v_part(b, q):
                # V rows for key chunks 2q, 2q+1: 16 MMs + 1 copy
                ps = psA.tile([128, 2 * MT], fp32, tag="big", name=f"vp{b}{q}")
                for j in range(2):
                    kc = 2 * q + j
                    for cc in range(CC):
                        nc.tensor.matmul(
                            ps[:, j * 128:(j + 1) * 128],
                            lhsT=xch[("r", b, cc)][:, kc * 128:(kc + 1) * 128],
                            rhs=wv_sb[:, cc, :],
                            start=(cc == 0),
                            stop=(cc == CC - 1),
                        )
                nc.vector.tensor_copy(
                    vA[b][:, 2 * q:2 * q + 2, :, 0:HD],
                    ps[:, 0:256].rearrange("p (k h d) -> p k h d", k=2, h=HPC),
                )

            scale = float(HD) ** -0.5

            def attn_mt(b, mt, fillers=None, av_lag=1, tail=(), scalar_cast=False):
                fillers = fillers or {}
                # for the LAST m-tile the exp stream is over, so its output
                # casts can run on the then-idle scalar engine, shortening
                # the last-exp -> a2a-fence chain (vector would serialize)
                cast = nc.scalar.copy if scalar_cast else nc.vector.tensor_copy
                po = [
                    psO.tile([HD + 1, MT], fp32, tag="o", name=f"po{h}")
                    for h in range(HPC)
                ]

                def av_pair(kc, eS):
                    for h in range(HPC):
                        for j in range(2):
                            nc.tensor.matmul(
                                po[h][:],
                                lhsT=vA[b][:, kc + j, h, :],
                                rhs=eS[h][:, j, :],
                                start=(kc == 0 and j == 0),
                                stop=(kc == KC - 2 and j == 1),
                            )

                # software-pipelined by av_lag kc-pairs: the AV of pair k is
                # emitted AFTER the S^T/exp of pair k+av_lag (and any filler
                # PE work), so the PE always has wait-free work while the ACT
                # engine streams exps back-to-back (ACT is the bottleneck).
                pending = []
                for pi, kc in enumerate(range(0, KC, 2)):
                    pss = [
                        psA.tile([128, 2 * MT], fp32, tag="big", name="pss")
                        for _ in range(HPC)
                    ]
                    for j in range(2):
                        # the two heads sit at partitions 0-63 / 64-127 so the
                        # PE row-groups run their K=64 matmuls concurrently
                        for h in range(HPC):
                            nc.tensor.matmul(
                                pss[h][:, j * MT:(j + 1) * MT],
                                lhsT=kT[b][h * HD:(h + 1) * HD, (kc + j) * 128:(kc + j + 1) * 128],
                                rhs=qT[b][h * HD:(h + 1) * HD, mt * MT:(mt + 1) * MT],
                                start=True,
                                stop=True,
                            )
                    eS = [
                        epool.tile([128, 2, MT], bf16, tag="eS", name="eS")
                        for _ in range(HPC)
                    ]
                    for h in range(HPC):
                        nc.scalar.activation(
                            eS[h][:].rearrange("p a b -> p (a b)"),
                            pss[h][:],
                            AF.Exp,
                            scale=scale,
                        )
                    for f in fillers.get(pi, ()):
                        f()
                    if len(pending) == av_lag:
                        av_pair(*pending.pop(0))
                    pending.append((kc, eS))
                ti = 0
                while len(pending) > 1:
                    if ti < len(tail):
                        tail[ti]()
                        ti += 1
                    av_pair(*pending.pop(0))
                for f in tail[ti:]:
                    f()

                # final pair: per-head AV with the output/denominator casts
                # interleaved, so staging starts while the other head's AV
                # still runs (shortens the last-exp -> a2a-fence chain).
                # m-tile mt covers dst cores (mt%2)*4+q at column-half mt//2;
                # staging triggers alternate sync/gpsimd so the last m-tile's
                # six triggers don't serialize ~4us ahead of the a2a fence.
                hf = mt // 2
                s0 = (mt % 2) * 4
                kc_l, eS_l = pending.pop(0)
                for h in range(HPC):
                    for j in range(2):
                        nc.tensor.matmul(
                            po[h][:],
                            lhsT=vA[b][:, kc_l + j, h, :],
                            rhs=eS_l[h][:, j, :],
                            start=(kc_l == 0 and j == 0),
                            stop=(kc_l == KC - 2 and j == 1),
                        )
                    cast(
                        oU[b][h * HD:(h + 1) * HD, mt * MT:(mt + 1) * MT],
                        po[h][0:HD, :],
                    )
                    dt = spool.tile([HD + 1, MT], bf16, tag="dt", name="dt")
                    cast(dt[HD:HD + 1, :], po[h][HD:HD + 1, :])
                    [nc.sync, nc.gpsimd][h].dma_start(
                        a2a_in[b][s0:s0 + 4, CHPC + h:CHPC + h + 1, hf * HS:(hf + 1) * HS],
                        dt[HD:HD + 1, :].rearrange("a (q c) -> a q c", q=4),
                    )
                # the quarters span BOTH heads' partition rows, so they can
                # only be staged after the second head's oU cast
                for q in range(4):
                    [nc.sync, nc.gpsimd][q % 2].dma_start(
                        a2a_in[b][s0 + q][0:CHPC, hf * HS:(hf + 1) * HS],
                        oU[b][:, mt * MT + q * HS:mt * MT + (q + 1) * HS],
                    )

            def fire_a2a(b):
                nc.gpsimd.collective_compute(
                    "AllToAll",
                    mybir.AluOpType.bypass,
                    replica_groups=[list(range(NCORES))],
                    ins=[a2a_in[b][:].opt()],
                    outs=[a2a_out[b][:].opt()],
                )

            def proj_load(b, ldeng):
                # dsb rows 0-7 = even heads (h=0 of each src core), 8-15 = odd.
                # Allocated from the attention-path "dt" tag ON PURPOSE: the
                # pool WAR forces this load (and the reciprocal chain behind
                # it) after the attention dt casts, so the Tile scheduler can
                # never interleave a collective-gated wait in front of the
                # attention work on the vector queue.
                dsb = spool.tile([HD + 1, MT], bf16, tag="dt", name=f"dsb{b}")
                dsf = dsb[:].bitcast(fp32)  # fp32 view: gpsimd DMA casts on load
                for hh in range(2):
                    ldeng.dma_start(
                        dsf[hh * NCORES:(hh + 1) * NCORES, 0:MBLK],
                        a2a_out[b][:, CHPC + hh:CHPC + hh + 1, :].rearrange(
                            "i h m -> i (h m)"
                        ),
                    )
                # gather my m-block (all 1024 channels) in ONE trigger: the
                # permuted DRAM access pattern costs the same descriptors as
                # eight per-slot loads but 7 fewer ~650ns queue slots
                y_sb = ppool.tile([128, NCORES, MBLK], bf16, tag="y", name=f"y{b}")
                ldeng.dma_start(
                    y_sb[:],
                    a2a_out[b][:, 0:CHPC, :].rearrange("s p m -> p s m"),
                )
                rf = ppool.tile([16, MBLK], fp32, tag="rf", name=f"rf{b}")
                # ~51-ULP fast reciprocal (~5x cheaper than the iterative
                # divide); denominators are safely in [1, ~1e4]
                nc.vector.reciprocal_approx_fast(rf[:], dsf[0:16, 0:MBLK])
                rN = ppool.tile([16, MBLK], bf16, tag="rN", name=f"rN{b}")
                nc.vector.tensor_copy(rN[:], rf[:])
                ldeng.dma_start(
                    rdram[b].rearrange("h i m -> (h i) m"), rN[:]
                )
                rb = ppool.tile([128, NCORES, MBLK], bf16, tag="rb", name=f"rb{b}")
                for hh in range(2):
                    ldeng.dma_start(
                        rb[hh * HD:(hh + 1) * HD],
                        rdram[b, hh:hh + 1].to_broadcast((HD, NCORES, MBLK)),
                    )
                xn = ppool.tile([128, NCORES, MBLK], bf16, tag="xn", name=f"xn{b}")
                # two halves: the proj matmuls' cc 0-3 start on the first
                # half while the second is still multiplying
                for g in range(2):
                    nc.vector.tensor_mul(
                        xn[:, 4 * g:4 * g + 4, :],
                        y_sb[:, 4 * g:4 * g + 4, :],
                        rb[:, 4 * g:4 * g + 4, :],
                    )
                return xn

            def proj_mm(b, xn):
                for oc in range(CC):
                    psb = psA.tile([128, 2 * MT], fp32, tag="big", name="pp")
                    ps = psb[:, 0:MBLK]
                    for cc in range(CC):
                        nc.tensor.matmul(
                            ps[:],
                            lhsT=wp_sb[:, cc, oc * 128:(oc + 1) * 128],
                            rhs=xn[:, cc, :],
                            start=(cc == 0),
                            stop=(cc == CC - 1),
                        )
                    osb = ppool.tile([128, MBLK], fp32, tag="outsb", name="osb")
                    nc.scalar.activation(
                        osb[:], ps[:], AF.Identity, bias=bias_sb[:, oc:oc + 1]
                    )
                    nc.sync.dma_start(out[b, oc * 128:(oc + 1) * 128, :], osb[:])

            # ================= emission schedule =================
            # batch-0 loads, halves-first so kt parts start after ~2MB of DMA
            E3 = [nc.sync, nc.gpsimd, nc.scalar]
            for half in range(2):
                for cc in range(CC):
                    load_chunk("r", 0, cc, half=half, eng=E3[cc % 3])
            for half in range(2):
                for cc in range(CC):
                    load_chunk("t", 0, cc, half=half, eng=E3[cc % 3])
            # wproj/bias after the batch-0 chunks on the scalar queue; needed
            # only by proj(0) mid-kernel
            wp_sb = wpool.tile([128, CC, C], bf16, name="wp_sb")
            for cc in range(CC):
                nc.scalar.dma_start(wp_sb[:, cc, :], wproj[cc * 128:(cc + 1) * 128, :])
            bias_sb = wpool.tile([128, CC], fp32, name="bias_sb")
            nc.scalar.dma_start(bias_sb[:], bproj.ap().rearrange("(a p) -> p a", p=128))

            # the minimum PE work before attention m-tile 0 can start: its
            # pairs 0-3 only need K^T keys 0-1023 and Q^T cols 0-511
            kt_part(0, 0, wk_sb, kT[0], "r", warm=3)
            kt_part(0, 0, wq_sb, qT[0], "t", warm=3)

            # everything else rides in attention-pair filler slots; av_lag=2
            # gives the just-in-time V parts one pair of slack
            P = functools.partial
            attn_mt(
                0, 0,
                fillers={
                    0: (P(kt_part, 0, 1, wk_sb, kT[0], "r"),),
                    1: (P(kt_part, 0, 2, wk_sb, kT[0], "r"),),
                    2: (P(kt_part, 0, 3, wk_sb, kT[0], "r"),),
                    3: (P(kt_part, 0, 1, wq_sb, qT[0], "t"),),
                    **{pi: (P(v_part, 0, pi - 4),) for pi in range(4, 8)},
                },
                av_lag=4,
                tail=(
                    P(v_part, 0, 4),
                    P(v_part, 0, 5),
                    P(v_part, 0, 6),
                    P(v_part, 0, 7),
                ),
            )
            # batch-1 chunk loads: emitted only now so their pool-recycling
            # waits (on r0/t0 release) sit behind this m-tile's staging in
            # the sync/gpsimd queues (scalar stays exp-only)
            E2 = [nc.sync, nc.gpsimd]
            for cc in range(CC):
                load_chunk("r", 1, cc, eng=E2[cc % 2])
            attn_mt(0, 1, {
                0: (functools.partial(kt_part, 0, 2, wq_sb, qT[0], "t"),),
                2: (functools.partial(kt_part, 0, 3, wq_sb, qT[0], "t"),),
            })
            for cc in range(CC):
                load_chunk("t", 1, cc, eng=E2[cc % 2])
            attn_mt(0, 2, {
                pi: (functools.partial(kt_part, 1, pi, wk_sb, kT[1], "r"),)
                for pi in range(4)
            })
            attn_mt(0, 3, {
                0: (functools.partial(kt_part, 1, 0, wq_sb, qT[1], "t"),),
                2: (functools.partial(kt_part, 1, 1, wq_sb, qT[1], "t"),),
            })
            fire_a2a(0)
            attn_mt(1, 0, {pi: (functools.partial(v_part, 1, pi),) for pi in range(8)})
            attn_mt(1, 1, {
                0: (functools.partial(kt_part, 1, 2, wq_sb, qT[1], "t"),),
                2: (functools.partial(kt_part, 1, 3, wq_sb, qT[1], "t"),),
            })
            attn_mt(1, 2)
            # proj(0)'s loads + reciprocal chain are emitted here so their
            # gpsimd triggers sit BEFORE the a2a(1) trigger (whose fence
            # would otherwise hold them hostage until all staging lands);
            # the dsb pool-WAR still pins the vector chain safely behind
            # attn(1,1)'s dt casts.  Its MATMULS stay after attn(1,3) so
            # the PE queue is never head-of-line blocked -- and they double
            # as the HAM warm-keeper spanning the a2a(1) wait.
            xn0 = proj_load(0, nc.gpsimd)
            attn_mt(1, 3, scalar_cast=True)
            fire_a2a(1)
            proj_mm(0, xn0)
            xn1 = proj_load(1, nc.gpsimd)
            # a short HAM warm-keeper after proj(0): spans the early part of
            # the a2a(1) wait so proj(1)'s matmuls are less likely to run at
            # the re-throttled clock; sized to finish before xn1 is ready
            # even when the collective is fast.
            for wg in range(4):
                wps = psA.tile([128, 2 * MT], fp32, tag="big", name=f"warm{wg}")
                for wi in range(16):
                    nc.tensor.matmul(
                        wps[:, 0:MT],
                        lhsT=wk_sb[:, wi % CC, :],
                        rhs=qT[1][:, 0:MT],
                        start=(wi == 0),
                        stop=(wi == 15),
                    )
            proj_mm(1, xn1)

    nc.compile()
    return nc


def _shard_inputs(reference_data, target_data, Wq, Wkv, Wproj, bproj):
    import ml_dtypes

    bf16 = ml_dtypes.bfloat16
    xrefT = np.ascontiguousarray(
        np.asarray(reference_data, dtype=np.float32).transpose(0, 2, 1)
    ).astype(bf16)
    xtgtT = np.ascontiguousarray(
        np.asarray(target_data, dtype=np.float32).transpose(0, 2, 1)
    ).astype(bf16)
    Wq = np.asarray(Wq, dtype=np.float32)
    Wkv = np.asarray(Wkv, dtype=np.float32)
    Wproj_b = np.asarray(Wproj, dtype=np.float32).astype(bf16)
    bproj = np.asarray(bproj, dtype=np.float32)

    in_maps = []
    for c in range(NCORES):
        lo, hi = c * CHPC, (c + 1) * CHPC
        in_maps.append(
            {
                "xrefT": xrefT,
                "xtgtT": xtgtT,
                "wq": Wq[:, lo:hi].astype(bf16),
                "wk": Wkv[:, lo:hi].astype(bf16),
                "wv": Wkv[:, C + lo:C + hi].astype(bf16),
                "wproj": Wproj_b,
                "bproj": bproj,
            }
        )
    return in_maps


def _ensure_ntff_hook():
    """Register the axon NTFF profile hook if the image's antenv lacks it."""
    try:
        import antenv.axon_hooks  # noqa: F401

        return
    except ImportError:
        pass
    import sys
    import types

    import antenv

    mod = types.ModuleType("antenv.axon_hooks")
    state = {"hook": None}
    mod.set_axon_ntff_profile_hook = lambda h: state.__setitem__("hook", h)
    mod.get_axon_ntff_profile_hook = lambda: state["hook"]
    sys.modules["antenv.axon_hooks"] = mod
    antenv.axon_hooks = mod
    try:
        from trn_agent_boot.trn_boot import _ntff_profile_via_ctypes

        mod.set_axon_ntff_profile_hook(
            _ntff_profile_via_ctypes("/opt/axon/libaxon_pjrt.so")
        )
    except Exception:
        pass


def run(inputs: dict, trace: bool = False):
    """Compile (cached), run on 8 cores, return (full_output, BassKernelResults)."""
    from concourse.bass_utils import run_bass_kernel_spmd

    if trace:
        _ensure_ntff_hook()
    nc = _build()
    in_maps = _shard_inputs(**inputs)
    res = run_bass_kernel_spmd(
        nc, in_maps, core_ids=list(range(NCORES)), trace=trace
    )
    return _assemble(res), res


def _assemble(res):
    full = np.zeros((B, M, C), dtype=np.float32)
    hs = MBLK // 2
    for c in range(NCORES):
        blk = np.asarray(res.results[c]["out"], dtype=np.float32)  # [B, C, MBLK]
        for b in range(B):
            for hf in range(2):
                full[b, 1024 * hf + c * hs:1024 * hf + (c + 1) * hs, :] = (
                    blk[b][:, hf * hs:(hf + 1) * hs].T
                )
    return full


def kernel(reference_data, target_data, Wq, Wkv, Wproj, bproj) -> np.ndarray:
    full, _ = run(
        {
            "reference_data": reference_data,
            "target_data": target_data,
            "Wq": Wq,
            "Wkv": Wkv,
            "Wproj": Wproj,
            "bproj": bproj,
        }
    )
    return full


# revision 48
# speedup vs baseline: 1.0721x; 1.0343x over previous
"""Distributed attention kernel for 8 TRN2 NeuronCores.

Problem: cross-attention (q from target, k/v from reference) with
B=2, N=M=2048, C=1024, H=16 heads, hd=64, followed by an output
projection with bias.

Sharding (data + head parallel):
  core c in 0..7 owns heads {2c, 2c+1} for BOTH batches. It computes
  K^T/Q^T/V for its heads and attention (softmax over keys), producing
  UNNORMALIZED x_local^T [128ch, 2048m] per batch plus per-head softmax
  denominators. One AllToAll PER BATCH redistributes [130, 256] blocks
  (128 channels + 2 denominator rows) so core c owns output rows
  [c*256, (c+1)*256) with ALL 1024 channels; core c then normalizes
  (one reciprocal + broadcast-multiply per batch) and applies the full
  Wproj ([1024,1024], replicated) + bias to its row-block.

Schedule: K^T and the first half of Q^T are computed first so the
softmax exp stream (the ScalarE roofline of this kernel, ~147us) starts
as early as possible; V, the rest of Q, and ALL of batch-1's QKV are
emitted as "fillers" between attention kc-pairs so the PE does them in
the gaps while ACT streams exps back-to-back.

Queue discipline (3 DMA-trigger queues: sync, scalar, gpsimd):
  - scalar carries only pre-attention input loads, then exps: nothing
    pool- or collective-gated may sit in front of an exp.
  - sync carries input chunks + per-m-tile staging into the a2a input.
  - gpsimd carries input chunks, the collective triggers, and all
    proj-side loads (which wait on the collectives) so a slow AllToAll
    can never head-of-line-block the attention/staging path.

Matmuls run in bf16 (f32 PSUM accumulation); softmax denominators come
free as a ones-column appended to V.
"""

import functools

import numpy as np

B = 2
N = 2048  # reference rows (keys)
M = 2048  # target rows (queries)
C = 1024
H = 16
HD = 64
NCORES = 8
HPC = 2  # heads per core
CHPC = HPC * HD  # 128 channels per core
CHP2 = CHPC + 2  # + 2 denominator rows in the a2a payload
MBLK = M // NCORES  # 256 output rows owned per core (per batch)
MT = 512  # attention m-tile
KC = N // 128  # 16 key chunks
CC = C // 128  # 8 contraction chunks
NMT = M // MT  # 4 m-tiles per batch
HS = MBLK // 2  # 128-row half-slot


@functools.lru_cache(maxsize=1)
def _build():
    import concourse.bacc as bacc
    import concourse.mybir as mybir
    import concourse.tile as tile

    fp32 = mybir.dt.float32
    bf16 = mybir.dt.bfloat16
    AF = mybir.ActivationFunctionType

    nc = bacc.Bacc("TRN2", target_bir_lowering=False, debug=False, num_devices=NCORES)

    xrefT = nc.dram_tensor("xrefT", [B, C, N], bf16, kind="ExternalInput")
    xtgtT = nc.dram_tensor("xtgtT", [B, C, M], bf16, kind="ExternalInput")
    wq = nc.dram_tensor("wq", [C, CHPC], bf16, kind="ExternalInput")
    wk = nc.dram_tensor("wk", [C, CHPC], bf16, kind="ExternalInput")
    wv = nc.dram_tensor("wv", [C, CHPC], bf16, kind="ExternalInput")
    wproj = nc.dram_tensor("wproj", [C, C], bf16, kind="ExternalInput")
    bproj = nc.dram_tensor("bproj", [C], fp32, kind="ExternalInput")
    out = nc.dram_tensor("out", [B, C, MBLK], fp32, kind="ExternalOutput")

    with tile.TileContext(nc) as tc:
        with (
            tc.tile_pool(name="wpool", bufs=1) as wpool,
            tc.tile_pool(name="xr", bufs=10) as xrpool,
            tc.tile_pool(name="xt", bufs=10) as xtpool,
            tc.tile_pool(name="kqv", bufs=1) as kqv,
            tc.tile_pool(name="epool", bufs=10) as epool,
            tc.tile_pool(name="spool", bufs=4) as spool,
            tc.tile_pool(name="ppool", bufs=2) as ppool,
            tc.tile_pool(name="psA", bufs=3, space="PSUM") as psA,
            tc.tile_pool(name="psO", bufs=2, space="PSUM") as psO,
            tc.tile_pool(name="dram", bufs=1, space="DRAM") as dpool,
        ):
            # ---- PE warm-up: ~48 matmuls on an uninitialized scratch ----
            # tile (results unread).  They stream while the input DMAs ramp,
            # so the HAM clock-gate is already at 2.4 GHz when the first
            # real QKV matmul arrives (~20us in) instead of half-clock.
            scratch = wpool.tile([128, 5 * 128], bf16, name="warmscratch")
            nc.vector.memset(scratch[:], 0.0)
            for wg in range(3):
                wps = psA.tile([128, 2 * MT], fp32, tag="big", name=f"wu{wg}")
                for wi in range(16):
                    nc.tensor.matmul(
                        wps[:, 0:MT],
                        lhsT=scratch[:, 0:128],
                        rhs=scratch[:, 128:5 * 128],
                        start=(wi == 0),
                        stop=(wi == 15),
                    )

            # ---- weight loads (emitted first; DMA engines run ahead) ----
            wq_sb = wpool.tile([128, CC, CHPC], bf16)
            wk_sb = wpool.tile([128, CC, CHPC], bf16)
            wv_sb = wpool.tile([128, CC, CHPC], bf16)
            for cc in range(CC):
                nc.sync.dma_start(wk_sb[:, cc, :], wk[cc * 128:(cc + 1) * 128, :])
                nc.gpsimd.dma_start(wv_sb[:, cc, :], wv[cc * 128:(cc + 1) * 128, :])
                nc.scalar.dma_start(wq_sb[:, cc, :], wq[cc * 128:(cc + 1) * 128, :])

            kT = [kqv.tile([128, N], bf16, tag=f"kT{b}", name=f"kT{b}") for b in range(B)]
            qT = [kqv.tile([128, M], bf16, tag=f"qT{b}", name=f"qT{b}") for b in range(B)]
            vA = [
                kqv.tile([128, KC, HPC, HD + 1], bf16, tag=f"vA{b}", name=f"vA{b}")
                for b in range(B)
            ]
            oU = [kqv.tile([128, M], bf16, tag=f"oU{b}", name=f"oU{b}") for b in range(B)]
            for b in range(B):
                nc.vector.memset(vA[b][:, :, :, HD:HD + 1], 1.0)

            a2a_in = [
                dpool.tile([NCORES, CHP2, MBLK], bf16, tag=f"a2a_in{b}", name=f"a2a_in{b}")
                for b in range(B)
            ]
            a2a_out = [
                dpool.tile([NCORES, CHP2, MBLK], bf16, tag=f"a2a_out{b}", name=f"a2a_out{b}")
                for b in range(B)
            ]
            # per-batch bounce for the reciprocal'd denominators, indexed
            # [h-within-pair, src-core, m]: row (hh, i) is head 2i+hh, so a
            # [1, 8, m] slice broadcasts across each 64-partition head group
            rdram = dpool.tile([B, 2, NCORES, MBLK], bf16, tag="rdram", name="rdram")

            xch = {}  # (tensor, b, cc) -> sbuf chunk tile

            def load_chunk(which, b, cc, half=None, eng=None):
                src = xrefT if which == "r" else xtgtT
                pool = xrpool if which == "r" else xtpool
                key = (which, b, cc)
                if key in xch:
                    t = xch[key]
                else:
                    t = pool.tile([128, N], bf16, tag="x", name=f"x{which}{b}_{cc}")
                    xch[key] = t
                if half is None:
                    eng.dma_start(t[:], src[b, cc * 128:(cc + 1) * 128, :])
                else:
                    cols = slice(half * (N // 2), (half + 1) * (N // 2))
                    eng.dma_start(t[:, cols], src[b, cc * 128:(cc + 1) * 128, cols])

            def kt_part(b, nt, w_sb, dstT, which):
                # one 512-col slice of a K^T/Q^T projection: 8 MMs + 1 copy
                ps = psA.tile([128, 2 * MT], fp32, tag="big", name=f"kp{which}{b}{nt}")
                for cc in range(CC):
                    nc.tensor.matmul(
                        ps[:, 0:MT],
                        lhsT=w_sb[:, cc, :],
                        rhs=xch[(which, b, cc)][:, nt * MT:(nt + 1) * MT],
                        start=(cc == 0),
                        stop=(cc == CC - 1),
                    )
                nc.vector.tensor_copy(dstT[:, nt * MT:(nt + 1) * MT], ps[:, 0:MT])

            def kt_split(b, nt, w_sb, dstT, which):
                # kt_part split across TWO filler slots (same single psum
                # tile, one accumulation group): halves the PE work inserted
                # between a pair's scores and AV, shrinking the exp gaps in
                # kt-filler m-tiles.  Allocation-count neutral vs kt_part.
                cell = {}

                def half_a():
                    cell["ps"] = psA.tile(
                        [128, 2 * MT], fp32, tag="big", name=f"ks{which}{b}{nt}"
                    )
                    for cc in range(CC // 2):
                        nc.tensor.matmul(
                            cell["ps"][:, 0:MT],
                            lhsT=w_sb[:, cc, :],
                            rhs=xch[(which, b, cc)][:, nt * MT:(nt + 1) * MT],
                            start=(cc == 0),
                            stop=False,
                        )

                def half_b():
                    for cc in range(CC // 2, CC):
                        nc.tensor.matmul(
                            cell["ps"][:, 0:MT],
                            lhsT=w_sb[:, cc, :],
                            rhs=xch[(which, b, cc)][:, nt * MT:(nt + 1) * MT],
                            start=False,
                            stop=(cc == CC - 1),
                        )
                    nc.vector.tensor_copy(
                        dstT[:, nt * MT:(nt + 1) * MT], cell["ps"][:, 0:MT]
                    )

                return half_a, half_b

            def v_part(b, q):
                # V rows for key chunks 2q, 2q+1: 16 MMs + 1 copy
                ps = psA.tile([128, 2 * MT], fp32, tag="big", name=f"vp{b}{q}")
                for j in range(2):
                    kc = 2 * q + j
                    for cc in range(CC):
                        nc.tensor.matmul(
                            ps[:, j * 128:(j + 1) * 128],
                            lhsT=xch[("r", b, cc)][:, kc * 128:(kc + 1) * 128],
                            rhs=wv_sb[:, cc, :],
                            start=(cc == 0),
                            stop=(cc == CC - 1),
                        )
                nc.vector.tensor_copy(
                    vA[b][:, 2 * q:2 * q + 2, :, 0:HD],
                    ps[:, 0:256].rearrange("p (k h d) -> p k h d", k=2, h=HPC),
                )

            scale = float(HD) ** -0.5

            def attn_mt(b, mt, fillers=None, av_lag=1, tail=(), scalar_cast=False):
                fillers = fillers or {}
                # for the LAST m-tile the exp stream is over, so its output
                # casts can run on the then-idle scalar engine, shortening
                # the last-exp -> a2a-fence chain (vector would serialize)
                cast = nc.scalar.copy if scalar_cast else nc.vector.tensor_copy
                po = [
                    psO.tile([HD + 1, MT], fp32, tag="o", name=f"po{h}")
                    for h in range(HPC)
                ]

                def av_pair(kc, eS):
                    for h in range(HPC):
                        for j in range(2):
                            nc.tensor.matmul(
                                po[h][:],
                                lhsT=vA[b][:, kc + j, h, :],
                                rhs=eS[h][:, j, :],
                                start=(kc == 0 and j == 0),
                                stop=(kc == KC - 2 and j == 1),
                            )

                # software-pipelined by av_lag kc-pairs: the AV of pair k is
                # emitted AFTER the S^T/exp of pair k+av_lag (and any filler
                # PE work), so the PE always has wait-free work while the ACT
                # engine streams exps back-to-back (ACT is the bottleneck).
                pending = []
                for pi, kc in enumerate(range(0, KC, 2)):
                    pss = [
                        psA.tile([128, 2 * MT], fp32, tag="big", name="pss")
                        for _ in range(HPC)
                    ]
                    for j in range(2):
                        # the two heads sit at partitions 0-63 / 64-127 so the
                        # PE row-groups run their K=64 matmuls concurrently
                        for h in range(HPC):
                            nc.tensor.matmul(
                                pss[h][:, j * MT:(j + 1) * MT],
                                lhsT=kT[b][h * HD:(h + 1) * HD, (kc + j) * 128:(kc + j + 1) * 128],
                                rhs=qT[b][h * HD:(h + 1) * HD, mt * MT:(mt + 1) * MT],
                                start=True,
                                stop=True,
                            )
                    eS = [
                        epool.tile([128, 2, MT], bf16, tag="eS", name="eS")
                        for _ in range(HPC)
                    ]
                    for h in range(HPC):
                        nc.scalar.activation(
                            eS[h][:].rearrange("p a b -> p (a b)"),
                            pss[h][:],
                            AF.Exp,
                            scale=scale,
                        )
                    for f in fillers.get(pi, ()):
                        f()
                    if len(pending) == av_lag:
                        av_pair(*pending.pop(0))
                    pending.append((kc, eS))
                ti = 0
                while len(pending) > 1:
                    if ti < len(tail):
                        tail[ti]()
                        ti += 1
                    av_pair(*pending.pop(0))
                for f in tail[ti:]:
                    f()

                # final pair: per-head AV with the output/denominator casts
                # interleaved, so staging starts while the other head's AV
                # still runs (shortens the last-exp -> a2a-fence chain).
                # m-tile mt covers dst cores (mt%2)*4+q at column-half mt//2;
                # staging triggers alternate sync/gpsimd so the last m-tile's
                # six triggers don't serialize ~4us ahead of the a2a fence.
                hf = mt // 2
                s0 = (mt % 2) * 4
                kc_l, eS_l = pending.pop(0)
                for h in range(HPC):
                    for j in range(2):
                        nc.tensor.matmul(
                            po[h][:],
                            lhsT=vA[b][:, kc_l + j, h, :],
                            rhs=eS_l[h][:, j, :],
                            start=(kc_l == 0 and j == 0),
                            stop=(kc_l == KC - 2 and j == 1),
                        )
                    cast(
                        oU[b][h * HD:(h + 1) * HD, mt * MT:(mt + 1) * MT],
                        po[h][0:HD, :],
                    )
                    dt = spool.tile([HD + 1, MT], bf16, tag="dt", name="dt")
                    cast(dt[HD:HD + 1, :], po[h][HD:HD + 1, :])
                    [nc.sync, nc.gpsimd][h].dma_start(
                        a2a_in[b][s0:s0 + 4, CHPC + h:CHPC + h + 1, hf * HS:(hf + 1) * HS],
                        dt[HD:HD + 1, :].rearrange("a (q c) -> a q c", q=4),
                    )
                # the quarters span BOTH heads' partition rows, so they can
                # only be staged after the second head's oU cast
                for q in range(4):
                    [nc.sync, nc.gpsimd][q % 2].dma_start(
                        a2a_in[b][s0 + q][0:CHPC, hf * HS:(hf + 1) * HS],
                        oU[b][:, mt * MT + q * HS:mt * MT + (q + 1) * HS],
                    )

            def fire_a2a(b):
                nc.gpsimd.collective_compute(
                    "AllToAll",
                    mybir.AluOpType.bypass,
                    replica_groups=[list(range(NCORES))],
                    ins=[a2a_in[b][:].opt()],
                    outs=[a2a_out[b][:].opt()],
                )

            def proj_load(b, ldeng):
                # dsb rows 0-7 = even heads (h=0 of each src core), 8-15 = odd.
                # Allocated from the attention-path "dt" tag ON PURPOSE: the
                # pool WAR forces this load (and the reciprocal chain behind
                # it) after the attention dt casts, so the Tile scheduler can
                # never interleave a collective-gated wait in front of the
                # attention work on the vector queue.
                dsb = spool.tile([HD + 1, MT], bf16, tag="dt", name=f"dsb{b}")
                dsf = dsb[:].bitcast(fp32)  # fp32 view: gpsimd DMA casts on load
                for hh in range(2):
                    ldeng.dma_start(
                        dsf[hh * NCORES:(hh + 1) * NCORES, 0:MBLK],
                        a2a_out[b][:, CHPC + hh:CHPC + hh + 1, :].rearrange(
                            "i h m -> i (h m)"
                        ),
                    )
                # gather my m-block (all 1024 channels) in ONE trigger: the
                # permuted DRAM access pattern costs the same descriptors as
                # eight per-slot loads but 7 fewer ~650ns queue slots
                y_sb = ppool.tile([128, NCORES, MBLK], bf16, tag="y", name=f"y{b}")
                ldeng.dma_start(
                    y_sb[:],
                    a2a_out[b][:, 0:CHPC, :].rearrange("s p m -> p s m"),
                )
                rf = ppool.tile([16, MBLK], fp32, tag="rf", name=f"rf{b}")
                # ~51-ULP fast reciprocal (~5x cheaper than the iterative
                # divide); denominators are safely in [1, ~1e4]
                nc.vector.reciprocal_approx_fast(rf[:], dsf[0:16, 0:MBLK])
                rN = ppool.tile([16, MBLK], bf16, tag="rN", name=f"rN{b}")
                nc.vector.tensor_copy(rN[:], rf[:])
                ldeng.dma_start(
                    rdram[b].rearrange("h i m -> (h i) m"), rN[:]
                )
                rb = ppool.tile([128, NCORES, MBLK], bf16, tag="rb", name=f"rb{b}")
                for hh in range(2):
                    ldeng.dma_start(
                        rb[hh * HD:(hh + 1) * HD],
                        rdram[b, hh:hh + 1].to_broadcast((HD, NCORES, MBLK)),
                    )
                xn = ppool.tile([128, NCORES, MBLK], bf16, tag="xn", name=f"xn{b}")
                # two halves: the proj matmuls' cc 0-3 start on the first
                # half while the second is still multiplying
                for g in range(2):
                    nc.vector.tensor_mul(
                        xn[:, 4 * g:4 * g + 4, :],
                        y_sb[:, 4 * g:4 * g + 4, :],
                        rb[:, 4 * g:4 * g + 4, :],
                    )
                return xn

            def proj_mm(b, xn):
                for oc in range(CC):
                    psb = psA.tile([128, 2 * MT], fp32, tag="big", name="pp")
                    ps = psb[:, 0:MBLK]
                    for cc in range(CC):
                        nc.tensor.matmul(
                            ps[:],
                            lhsT=wp_sb[:, cc, oc * 128:(oc + 1) * 128],
                            rhs=xn[:, cc, :],
                            start=(cc == 0),
                            stop=(cc == CC - 1),
                        )
                    osb = ppool.tile([128, MBLK], fp32, tag="outsb", name="osb")
                    nc.scalar.activation(
                        osb[:], ps[:], AF.Identity, bias=bias_sb[:, oc:oc + 1]
                    )
                    nc.sync.dma_start(out[b, oc * 128:(oc + 1) * 128, :], osb[:])

            # ================= emission schedule =================
            # batch-0 loads, halves-first so kt parts start after ~2MB of DMA
            E3 = [nc.sync, nc.gpsimd, nc.scalar]
            for half in range(2):
                for cc in range(CC):
                    load_chunk("r", 0, cc, half=half, eng=E3[cc % 3])
            for half in range(2):
                for cc in range(CC):
                    load_chunk("t", 0, cc, half=half, eng=E3[cc % 3])
            # wproj/bias after the batch-0 chunks on the scalar queue; needed
            # only by proj(0) mid-kernel
            wp_sb = wpool.tile([128, CC, C], bf16, name="wp_sb")
            for cc in range(CC):
                nc.scalar.dma_start(wp_sb[:, cc, :], wproj[cc * 128:(cc + 1) * 128, :])
            bias_sb = wpool.tile([128, CC], fp32, name="bias_sb")
            nc.scalar.dma_start(bias_sb[:], bproj.ap().rearrange("(a p) -> p a", p=128))

            # the minimum PE work before attention m-tile 0 can start: its
            # pairs 0-3 only need K^T keys 0-1023 and Q^T cols 0-511
            kt_part(0, 0, wk_sb, kT[0], "r")
            kt_part(0, 0, wq_sb, qT[0], "t")

            # everything else rides in attention-pair filler slots; av_lag=2
            # gives the just-in-time V parts one pair of slack
            P = functools.partial
            attn_mt(
                0, 0,
                fillers={
                    0: (P(kt_part, 0, 1, wk_sb, kT[0], "r"),),
                    1: (P(kt_part, 0, 2, wk_sb, kT[0], "r"),),
                    2: (P(kt_part, 0, 3, wk_sb, kT[0], "r"),),
                    3: (P(kt_part, 0, 1, wq_sb, qT[0], "t"),),
                    **{pi: (P(v_part, 0, pi - 4),) for pi in range(4, 8)},
                },
                av_lag=4,
                tail=(
                    P(v_part, 0, 4),
                    P(v_part, 0, 5),
                    P(v_part, 0, 6),
                    P(v_part, 0, 7),
                ),
            )
            # batch-1 chunk loads: emitted only now so their pool-recycling
            # waits (on r0/t0 release) sit behind this m-tile's staging in
            # the sync/gpsimd queues (scalar stays exp-only)
            E2 = [nc.sync, nc.gpsimd]
            for cc in range(CC):
                load_chunk("r", 1, cc, eng=E2[cc % 2])
            q2a, q2b = kt_split(0, 2, wq_sb, qT[0], "t")
            q3a, q3b = kt_split(0, 3, wq_sb, qT[0], "t")
            attn_mt(0, 1, {0: (q2a,), 1: (q2b,), 2: (q3a,), 3: (q3b,)})
            for cc in range(CC):
                load_chunk("t", 1, cc, eng=E2[cc % 2])
            k1s = [kt_split(1, p, wk_sb, kT[1], "r") for p in range(4)]
            attn_mt(0, 2, {2 * p + i: (k1s[p][i],) for p in range(4) for i in range(2)})
            q10 = kt_split(1, 0, wq_sb, qT[1], "t")
            q11 = kt_split(1, 1, wq_sb, qT[1], "t")
            attn_mt(0, 3, {0: (q10[0],), 1: (q10[1],), 2: (q11[0],), 3: (q11[1],)})
            fire_a2a(0)
            attn_mt(1, 0, {pi: (functools.partial(v_part, 1, pi),) for pi in range(8)})
            q12 = kt_split(1, 2, wq_sb, qT[1], "t")
            q13 = kt_split(1, 3, wq_sb, qT[1], "t")
            attn_mt(1, 1, {0: (q12[0],), 1: (q12[1],), 2: (q13[0],), 3: (q13[1],)})
            attn_mt(1, 2)
            # proj(0)'s loads + reciprocal chain are emitted here so their
            # gpsimd triggers sit BEFORE the a2a(1) trigger (whose fence
            # would otherwise hold them hostage until all staging lands);
            # the dsb pool-WAR still pins the vector chain safely behind
            # attn(1,1)'s dt casts.  Its MATMULS stay after attn(1,3) so
            # the PE queue is never head-of-line blocked -- and they double
            # as the HAM warm-keeper spanning the a2a(1) wait.
            xn0 = proj_load(0, nc.gpsimd)
            attn_mt(1, 3, scalar_cast=True)
            fire_a2a(1)
            proj_mm(0, xn0)
            xn1 = proj_load(1, nc.gpsimd)
            # a short HAM warm-keeper after proj(0): spans the early part of
            # the a2a(1) wait so proj(1)'s matmuls are less likely to run at
            # the re-throttled clock; sized to finish before xn1 is ready
            # even when the collective is fast.
            for wg in range(4):
                wps = psA.tile([128, 2 * MT], fp32, tag="big", name=f"warm{wg}")
                for wi in range(16):
                    nc.tensor.matmul(
                        wps[:, 0:MT],
                        lhsT=wk_sb[:, wi % CC, :],
                        rhs=qT[1][:, 0:MT],
                        start=(wi == 0),
                        stop=(wi == 15),
                    )
            proj_mm(1, xn1)

    nc.compile()
    return nc


def _shard_inputs(reference_data, target_data, Wq, Wkv, Wproj, bproj):
    import ml_dtypes

    bf16 = ml_dtypes.bfloat16
    xrefT = np.ascontiguousarray(
        np.asarray(reference_data, dtype=np.float32).transpose(0, 2, 1)
    ).astype(bf16)
    xtgtT = np.ascontiguousarray(
        np.asarray(target_data, dtype=np.float32).transpose(0, 2, 1)
    ).astype(bf16)
    Wq = np.asarray(Wq, dtype=np.float32)
    Wkv = np.asarray(Wkv, dtype=np.float32)
    Wproj_b = np.asarray(Wproj, dtype=np.float32).astype(bf16)
    bproj = np.asarray(bproj, dtype=np.float32)

    in_maps = []
    for c in range(NCORES):
        lo, hi = c * CHPC, (c + 1) * CHPC
        in_maps.append(
            {
                "xrefT": xrefT,
                "xtgtT": xtgtT,
                "wq": Wq[:, lo:hi].astype(bf16),
                "wk": Wkv[:, lo:hi].astype(bf16),
                "wv": Wkv[:, C + lo:C + hi].astype(bf16),
                "wproj": Wproj_b,
                "bproj": bproj,
            }
        )
    return in_maps


def _ensure_ntff_hook():
    """Register the axon NTFF profile hook if the image's antenv lacks it."""
    try:
        import antenv.axon_hooks  # noqa: F401

        return
    except ImportError:
        pass
    import sys
    import types

    import antenv

    mod = types.ModuleType("antenv.axon_hooks")
    state = {"hook": None}
    mod.set_axon_ntff_profile_hook = lambda h: state.__setitem__("hook", h)
    mod.get_axon_ntff_profile_hook = lambda: state["hook"]
    sys.modules["antenv.axon_hooks"] = mod
    antenv.axon_hooks = mod
    try:
        from trn_agent_boot.trn_boot import _ntff_profile_via_ctypes

        mod.set_axon_ntff_profile_hook(
            _ntff_profile_via_ctypes("/opt/axon/libaxon_pjrt.so")
        )
    except Exception:
        pass


def run(inputs: dict, trace: bool = False):
    """Compile (cached), run on 8 cores, return (full_output, BassKernelResults)."""
    from concourse.bass_utils import run_bass_kernel_spmd

    if trace:
        _ensure_ntff_hook()
    nc = _build()
    in_maps = _shard_inputs(**inputs)
    res = run_bass_kernel_spmd(
        nc, in_maps, core_ids=list(range(NCORES)), trace=trace
    )
    return _assemble(res), res


def _assemble(res):
    full = np.zeros((B, M, C), dtype=np.float32)
    hs = MBLK // 2
    for c in range(NCORES):
        blk = np.asarray(res.results[c]["out"], dtype=np.float32)  # [B, C, MBLK]
        for b in range(B):
            for hf in range(2):
                full[b, 1024 * hf + c * hs:1024 * hf + (c + 1) * hs, :] = (
                    blk[b][:, hf * hs:(hf + 1) * hs].T
                )
    return full


def kernel(reference_data, target_data, Wq, Wkv, Wproj, bproj) -> np.ndarray:
    full, _ = run(
        {
            "reference_data": reference_data,
            "target_data": target_data,
            "Wq": Wq,
            "Wkv": Wkv,
            "Wproj": Wproj,
            "bproj": bproj,
        }
    )
    return full
